# revision 1
# baseline (speedup 1.0000x reference)
"""Multi-head attention (RoPE, causal) Trainium2 Bass kernel.

Problem: nn_MultiHeadAttention_62431644615193
  x:     [2, 2048, 1024] f32
  mask:  [1, 1, 2048, 2048] i32 (causal tril expected)
  w_qkv: [1024, 3072] f32
  w_out: [1024, 1024] f32
  out:   [2, 2048, 1024] f32

Sharding over 8 cores: data-parallel on batch (2) x tensor-parallel on
heads (16 heads -> 4 per core). Each core computes a partial output
[2048, 1024] (its heads' contribution through w_out rows); the host sums
the 4 partials per batch.

Per-core dataflow (all matmuls float32r = full PE rate):
  1. qT,kT projection transposed:  qkT[c, s] = w_qk^T @ x^T   (c on partitions)
  2. v projection natural:          v[t, e]  = (x^T chunk as lhsT) @ w_v
  3. RoPE via small permutation matmul + DVE elementwise combine
  4. attention, scores transposed:  pT[t, s] = kT_blk^T-slice @ qT  (+ -1e9 mask
     matmul on diagonal blocks), ACT exp PSUM->SBUF, PV with ones-augmented V
     giving outT[e, s] rows 0-63 and the softmax denominator replicated on
     rows 64-127; normalize with reciprocal_approx_fast + tensor_mul
  5. out-projection: lhsT = attn_outT chunks, rhs = w_out rows for the core's
     heads -> partial [2048, 1024] streamed out per 128-row tile
"""

import math

import numpy as np

import concourse.bass as bass
import concourse.tile as tile
from concourse import bacc
import concourse.mybir as mybir
from concourse.bass_utils import run_bass_kernel_spmd

B, S, D = 2, 2048, 1024
H = 16
HD = D // H          # 64
HPC = H // 4         # 4 heads per core
ROPE_BASE = 10000.0

F32 = mybir.dt.float32
F32R = mybir.dt.float32r
AF = mybir.ActivationFunctionType

NEG = -1.0e9


# --------------------------------------------------------------------------
# bass program (shared by all 8 cores; per-core data differs)
# --------------------------------------------------------------------------

def build_nc(causal: bool = True, reps: int = 1):
    nc = bacc.Bacc("TRN2", target_bir_lowering=False, debug=False, num_devices=8)

    xT = nc.dram_tensor("xT", [D, S], F32, kind="ExternalInput")
    w_qk = nc.dram_tensor("w_qk", [D, 8 * HD], F32, kind="ExternalInput")
    w_v = nc.dram_tensor("w_v", [D, 4 * HD], F32, kind="ExternalInput")
    w_out = nc.dram_tensor("w_out", [4 * HD, D], F32, kind="ExternalInput")
    cos2 = nc.dram_tensor("cos2", [128, S], F32, kind="ExternalInput")
    sin2 = nc.dram_tensor("sin2", [128, S], F32, kind="ExternalInput")
    rotP = nc.dram_tensor("rotP", [128, 128], F32, kind="ExternalInput")
    ident = nc.dram_tensor("ident", [128, 128], F32, kind="ExternalInput")
    maskb = nc.dram_tensor("maskb", [128, 4 * 512], F32, kind="ExternalInput")
    mask01 = nc.dram_tensor("mask01", [128, 2 * 128], F32, kind="ExternalInput")
    onesd = nc.dram_tensor("onesd", [128, 64], F32, kind="ExternalInput")
    outp = nc.dram_tensor("outp", [S, D], F32, kind="ExternalOutput")

    NT = S // 128     # 16 t-blocks
    NI = 4            # fused quarter / attention s-chunks of 512

    with tile.TileContext(nc) as tc:
        with (
            tc.tile_pool(name="const", bufs=1) as cpool,
            tc.tile_pool(name="qkT", bufs=1) as qkTpool,
            tc.tile_pool(name="va", bufs=1) as vapool,
            tc.tile_pool(name="xq", bufs=2) as xqpool,
            tc.tile_pool(name="qkraw", bufs=4) as qkrawpool,
            tc.tile_pool(name="trig", bufs=2) as trigpool,
            tc.tile_pool(name="ropescratch", bufs=2) as rspool,
            tc.tile_pool(name="phat", bufs=5) as phatpool,
            tc.tile_pool(name="norm", bufs=3) as normpool,
            tc.tile_pool(name="attn_out", bufs=2) as aopool,
            tc.tile_pool(name="outstage", bufs=4) as ostpool,
            tc.tile_pool(name="ps", bufs=1, space="PSUM") as pspool,
        ):
            # ---------------- constants ----------------
            w_qk_t = [cpool.tile([128, 8 * HD], F32R, name=f"wqk{i}", tag=f"wqk{i}") for i in range(8)]
            w_v_t = [cpool.tile([128, 4 * HD], F32R, name=f"wv{i}", tag=f"wv{i}") for i in range(8)]
            w_out_t = [cpool.tile([128, D], F32R, name=f"wout{i}", tag=f"wout{i}") for i in range(2)]
            rotP_t = cpool.tile([128, 128], F32R)
            ident_t = cpool.tile([128, 128], F32R)
            maskb_t = cpool.tile([128, 4, 512], F32R)

            for dd in range(8):
                nc.sync.dma_start(
                    w_qk_t[dd][:], w_qk[128 * dd : 128 * dd + 128, :].bitcast(F32R)
                )
            for dd in range(8):
                nc.gpsimd.dma_start(
                    w_v_t[dd][:], w_v[128 * dd : 128 * dd + 128, :].bitcast(F32R)
                )
            for kk in range(2):
                nc.gpsimd.dma_start(
                    w_out_t[kk][:], w_out[128 * kk : 128 * kk + 128, :].bitcast(F32R)
                )
            nc.sync.dma_start(rotP_t[:], rotP[:].bitcast(F32R))
            nc.gpsimd.dma_start(ident_t[:], ident[:].bitcast(F32R))
            nc.gpsimd.dma_start(
                maskb_t[:], maskb[:].bitcast(F32R).rearrange("p (o s) -> p o s", o=4)
            )
            mask01_t = cpool.tile([128, 2, 128], F32)
            nc.gpsimd.dma_start(
                mask01_t[:], mask01[:].rearrange("p (b s) -> p b s", b=2)
            )

            # v_aug storage: per (t-block j, head h): [v_h(64) | 1] = 65 cols
            va_t = vapool.tile([128, NT, 4, HD + 1], F32R)
            nc.gpsimd.dma_start(
                va_t[:, :, :, HD : HD + 1],
                onesd[:].bitcast(F32R).rearrange("p (j h c) -> p j h c", j=NT, h=4),
            )

            # final rotated qT/kT: tiles [q_h0;q_h1], [k_h0;k_h1], [q_h2;q_h3], [k_h2;k_h3]
            qkT = [qkTpool.tile([128, S], F32R, name=f"qkT{i}", tag=f"qkT{i}") for i in range(4)]

            def load_proj(i):
                s_sl = slice(512 * i, 512 * i + 512)
                # ---- loads for this quarter ----
                xq = xqpool.tile([128, 8, 512], F32R, tag="xq", name="xq")
                for dd in range(8):
                    nc.sync.dma_start(
                        xq[:, dd, :],
                        xT[128 * dd : 128 * dd + 128, s_sl].bitcast(F32R),
                    )
                cos_q = trigpool.tile([128, 512], F32, tag="cos", name="cos_q")
                sin_q = trigpool.tile([128, 512], F32, tag="sin", name="sin_q")
                nc.sync.dma_start(cos_q[:], cos2[:, s_sl])
                nc.sync.dma_start(sin_q[:], sin2[:, s_sl])

                # ---- qk projection + rope for this quarter ----
                # all projection matmul groups first; the rot matmul for tile
                # mt only needs mt's PSUM->SBUF copy, which completes while
                # later tiles' matmuls run, so PE never stalls on the copy.
                qk_raws = []
                for mt in range(4):
                    ps = pspool.tile([128, 512], F32, tag="mm1", bufs=2, name="ps")
                    for dd in range(8):
                        nc.tensor.matmul(
                            ps[:],
                            w_qk_t[dd][:, 128 * mt : 128 * mt + 128],
                            xq[:, dd, :],
                            start=(dd == 0),
                            stop=(dd == 7),
                        )
                    qk_raw = qkrawpool.tile([128, 512], F32R, tag="qkraw", name="qk_raw")
                    nc.vector.tensor_copy(qk_raw[:], ps[:])
                    qk_raws.append(qk_raw)
                for mt in range(4):
                    qk_raw = qk_raws[mt]
                    psr = pspool.tile([128, 512], F32, tag="mm1", bufs=2, name="psr")
                    nc.tensor.matmul(psr[:], rotP_t[:], qk_raw[:], start=True, stop=True)
                    rotsin = rspool.tile([128, 512], F32, tag="rs", name="rotsin")
                    nc.vector.tensor_mul(rotsin[:], psr[:], sin_q[:])
                    qkcos = rspool.tile([128, 512], F32, tag="qkcos", name="qkcos")
                    nc.gpsimd.tensor_mul(qkcos[:], qk_raw[:].bitcast(F32), cos_q[:])
                    nc.vector.tensor_add(qkT[mt][:, s_sl], qkcos[:], rotsin[:])

                # ---- v projection for this quarter ----
                for st in range(4):
                    j = 4 * i + st
                    psv = pspool.tile([128, 4 * HD], F32, tag="mm1", bufs=2, name="psv")
                    for dd in range(8):
                        nc.tensor.matmul(
                            psv[:],
                            xq[:, dd, 128 * st : 128 * st + 128],
                            w_v_t[dd][:],
                            start=(dd == 0),
                            stop=(dd == 7),
                        )
                    nc.scalar.copy(
                        va_t[:, j, :, 0:HD], psv[:].rearrange("p (h c) -> p h c", h=4)
                    )

            def attention(i):
                s_sl = slice(512 * i, 512 * i + 512)
                nblk = 4 * i + 4 if causal else NT
                ao = [
                    aopool.tile([128, 512], F32R, tag="aot", name=f"ao{hp}")
                    for hp in range(2)
                ]
                for hp in range(2):
                    qt = qkT[2 * hp]
                    kt = qkT[2 * hp + 1]
                    ps_pv_e = pspool.tile([HD + 1, 512], F32, tag="pv", bufs=2, name="ps_pv_e")
                    ps_pv_o = pspool.tile([HD + 1, 512], F32, tag="pv", bufs=2, name="ps_pv_o")

                    def reg_of(j):
                        dvr = causal and 4 * i <= j
                        o = j - 4 * i if dvr else 0
                        # columns s' < 128*o of a diagonal block are fully
                        # masked -> restrict compute to [128*o : 512].
                        return dvr, o, slice(128 * o, 512)

                    def issue_qk(j):
                        dvr, o, reg = reg_of(j)
                        # fp32r needs moving dim >= 256 for full rate; for
                        # o == 3 compute qk full-width (same cycles).
                        qk_lo = reg.start if reg.start <= 256 else 0
                        ps_qk = pspool.tile([128, 2, 512], F32, tag="mm2", bufs=2, name="ps_qk")
                        for sl2 in range(2):
                            hb = 64 * sl2
                            nc.tensor.matmul(
                                ps_qk[:, sl2, qk_lo:512],
                                kt[hb : hb + 64, 128 * j : 128 * j + 128],
                                qt[hb : hb + 64, 512 * i + qk_lo : 512 * i + 512],
                                start=True,
                                stop=True,
                            )
                        phat = phatpool.tile([128, 2, 512], F32R, tag="phat", name="phat")
                        nc.scalar.activation(
                            phat[:, :, reg],
                            ps_qk[:, :, reg],
                            AF.Exp,
                            scale=1.0 / math.sqrt(HD),
                        )
                        if dvr:
                            # zero the masked triangle in the 128-wide strip
                            # where the causal boundary crosses this block
                            # (identical pattern for every offset o)
                            strip = slice(128 * o, 128 * o + 128)
                            nc.vector.tensor_mul(
                                phat[:, :, strip],
                                phat[:, :, strip].bitcast(F32),
                                mask01_t[:],
                            )
                        return phat

                    # software pipeline: issue qk/exp for j+1 before pv of j so
                    # PE (in-order) never idles waiting for exp.
                    phats = {0: issue_qk(0)}
                    for j in range(nblk):
                        if j + 1 < nblk:
                            phats[j + 1] = issue_qk(j + 1)
                        _, _, reg = reg_of(j)
                        phat = phats.pop(j)
                        nc.tensor.matmul(
                            ps_pv_e[:, reg],
                            va_t[:, j, 2 * hp, :],
                            phat[:, 0, reg],
                            start=(j == 0),
                            stop=(j == nblk - 1),
                        )
                        nc.tensor.matmul(
                            ps_pv_o[:, reg],
                            va_t[:, j, 2 * hp + 1, :],
                            phat[:, 1, reg],
                            start=(j == 0),
                            stop=(j == nblk - 1),
                        )
                    for sl2, ps_pv in ((0, ps_pv_e), (1, ps_pv_o)):
                        hb = 64 * sl2
                        den = normpool.tile([1, 512], F32, tag="den", name="den")
                        rec1 = normpool.tile([1, 512], F32, tag="rec1", name="rec1")
                        rec = normpool.tile([64, 512], F32, tag="rec", name="rec")
                        nc.scalar.copy(den[:], ps_pv[HD : HD + 1, :])
                        nc.vector.reciprocal_approx_fast(rec1[:], den[:])
                        nc.gpsimd.partition_broadcast(rec[:], rec1[:])
                        nc.vector.tensor_mul(
                            ao[hp][hb : hb + 64, :], ps_pv[0:HD, :], rec[:]
                        )
                return ao

            def outproj(i, ao):
                for st in range(4):
                    ssl = slice(512 * i + 128 * st, 512 * i + 128 * st + 128)
                    sloc = slice(128 * st, 128 * st + 128)
                    ostage = ostpool.tile([128, D], F32, tag="ost", name="ostage")
                    for n2 in range(2):
                        ps_o = pspool.tile([128, 512], F32, tag="pv", bufs=2, name="ps_o")
                        for kk in range(2):
                            nc.tensor.matmul(
                                ps_o[:],
                                ao[kk][:, sloc],
                                w_out_t[kk][:, 512 * n2 : 512 * n2 + 512],
                                start=(kk == 0),
                                stop=(kk == 1),
                            )
                        nc.vector.tensor_copy(
                            ostage[:, 512 * n2 : 512 * n2 + 512], ps_o[:]
                        )
                    nc.scalar.dma_start(outp[ssl, :], ostage[:])

            # causal: fused per-chunk pipeline (attention chunk i only needs
            # kT/v for t-blocks <= chunk end). non-causal: attention needs the
            # full kT/v, so project everything first.
            for _rep in range(reps):
                if causal:
                    ao_prev = None
                    for i in range(NI):
                        load_proj(i)
                        if ao_prev is not None:
                            outproj(i - 1, ao_prev)
                        ao_prev = attention(i)
                    outproj(NI - 1, ao_prev)
                else:
                    for i in range(NI):
                        load_proj(i)
                    for i in range(NI):
                        outproj(i, attention(i))

    nc.compile()
    return nc


# --------------------------------------------------------------------------
# host-side: constants, sharding, assembly
# --------------------------------------------------------------------------

def _rope_tables():
    inv_freq = 1.0 / (
        ROPE_BASE ** (np.arange(0, HD, 2, dtype=np.float32) / HD)
    )
    positions = np.arange(S, dtype=np.float32)
    freqs = np.outer(positions, inv_freq).astype(np.float32)     # [S, 32]
    emb = np.concatenate((freqs, freqs), axis=-1)                # [S, 64]
    cosT = np.cos(emb).T.astype(np.float32)                      # [64, S]
    sinT = np.sin(emb).T.astype(np.float32)
    cos2 = np.vstack([cosT, cosT]).copy()                        # [128, S]
    sin2 = np.vstack([sinT, sinT]).copy()
    return cos2, sin2


def _rot_lhsT():
    # rotate_half (interleaved): rot[2i] = -x[2i+1], rot[2i+1] = x[2i]
    # P[j, i]: rot[j] = sum_i P[j, i] x[i]; lhsT[i, j] = P[j, i]
    P = np.zeros((128, 128), np.float32)
    for base in (0, 64):
        for i2 in range(HD // 2):
            P[base + 2 * i2, base + 2 * i2 + 1] = -1.0
            P[base + 2 * i2 + 1, base + 2 * i2] = 1.0
    return P.T.copy()


def _mask_bias():
    # maskb[t, o, s'] = NEG where (t + 128*o) > s'  (within a 512 s-chunk,
    # for the 4 diagonal t-blocks at offsets o = j - 4i)
    t = np.arange(128)[:, None, None]
    o = np.arange(4)[None, :, None]
    sp = np.arange(512)[None, None, :]
    mb = np.where(t + 128 * o > sp, np.float32(NEG), np.float32(0.0))
    return mb.reshape(128, 4 * 512).astype(np.float32)


_CACHE: dict = {}


def _get_nc(causal: bool):
    key = ("nc", causal)
    if key not in _CACHE:
        _CACHE[key] = build_nc(causal)
    return _CACHE[key]


def _classify_mask(mask: np.ndarray) -> str:
    m = np.asarray(mask).reshape(S, S)
    if np.array_equal(m != 0, np.tril(np.ones((S, S), bool))):
        return "causal"
    if np.all(m != 0):
        return "full"
    return "other"


def make_in_maps(x, w_qkv, w_out):
    """Build the 8 per-core input dicts."""
    cos2, sin2 = _rope_tables()
    rotP = _rot_lhsT()
    maskb = _mask_bias()
    ident = np.eye(128, dtype=np.float32)
    # mask01[t, b, s''] = 0 where t > s'' (strict lower triangle masked), for
    # both head slots b
    m01 = (np.arange(128)[:, None] <= np.arange(128)[None, :]).astype(np.float32)
    mask01 = np.ascontiguousarray(np.stack([m01, m01], axis=1).reshape(128, 256))
    onesd = np.ones((128, 64), np.float32)

    w3 = np.asarray(w_qkv).reshape(D, 3, H, HD)   # [D, {q,k,v}, H, hd]
    wo = np.asarray(w_out)                        # [D, D]; rows indexed [h, hd]
    xT = [np.ascontiguousarray(np.asarray(x)[b].T) for b in range(B)]  # [D, S]

    in_maps = []
    for c in range(8):
        b, hg = divmod(c, 4)
        hs = [4 * hg + i for i in range(HPC)]
        # w_qk cols: [q_h0, q_h1, k_h0, k_h1, q_h2, q_h3, k_h2, k_h3]
        wqk_cols = []
        for pair in range(2):
            for t in range(2):  # 0 = q, 1 = k
                for hh in (hs[2 * pair], hs[2 * pair + 1]):
                    wqk_cols.append(w3[:, t, hh, :])
        w_qk_c = np.ascontiguousarray(np.concatenate(wqk_cols, axis=1))  # [D, 512]
        w_v_c = np.ascontiguousarray(
            np.concatenate([w3[:, 2, hh, :] for hh in hs], axis=1)
        )  # [D, 256]
        w_out_c = np.ascontiguousarray(
            np.concatenate([wo[HD * hh : HD * hh + HD, :] for hh in hs], axis=0)
        )  # [256, D]
        in_maps.append(
            {
                "xT": xT[b],
                "w_qk": w_qk_c,
                "w_v": w_v_c,
                "w_out": w_out_c,
                "cos2": cos2,
                "sin2": sin2,
                "rotP": rotP,
                "ident": ident,
                "maskb": maskb,
                "mask01": mask01,
                "onesd": onesd,
            }
        )
    return in_maps


def _reference_numpy(x, mask, w_qkv, w_out):
    """Exact fallback for non-causal, non-full masks (slow, host-side)."""
    x = np.asarray(x, np.float32)
    qkv = (x @ w_qkv).reshape(B, S, 3, H, HD)
    qkv = np.transpose(qkv, (2, 0, 3, 1, 4))
    q, k, v = qkv[0], qkv[1], qkv[2]
    cos2, sin2 = _rope_tables()
    cos = cos2[:HD].T[None, None]
    sin = sin2[:HD].T[None, None]

    def rot(t):
        t1 = t[..., ::2]
        t2 = t[..., 1::2]
        return np.stack((-t2, t1), axis=-1).reshape(t.shape)

    q = q * cos + rot(q) * sin
    k = k * cos + rot(k) * sin
    attn = np.einsum("bhsd,bhtd->bhst", q, k) / math.sqrt(HD)
    m = np.asarray(mask).reshape(1, 1, S, S)
    attn = np.where(m == 0, -np.inf, attn)
    attn = attn - attn.max(-1, keepdims=True)
    np.exp(attn, out=attn)
    attn /= attn.sum(-1, keepdims=True)
    out = np.einsum("bhst,bhtd->bhsd", attn, v)
    out = np.transpose(out, (0, 2, 1, 3)).reshape(B, S, D)
    return (out @ w_out).astype(np.float32)


class Runner:
    """Cached jitted SPMD runner (mirrors bass2jax.run_bass_via_pjrt)."""

    def __init__(self, nc, n_cores: int = 8):
        import jax
        import concourse.mybir as _mybir
        from concourse import bass2jax
        from jax.experimental.shard_map import shard_map
        from jax.sharding import Mesh, PartitionSpec

        bass2jax.install_neuronx_cc_hook()
        self.jax = jax
        self.n_cores = n_cores
        self._nc = nc
        in_names, out_names, out_avals, zero_outs = [], [], [], []
        for alloc in nc.m.functions[0].allocations:
            if not isinstance(alloc, _mybir.MemoryLocationSet):
                continue
            name = alloc.memorylocations[0].name
            if alloc.kind == "ExternalInput":
                in_names.append(name)
            elif alloc.kind == "ExternalOutput":
                out_names.append(name)
                shape = tuple(alloc.tensor_shape)
                dtype = _mybir.dt.np(alloc.dtype)
                out_avals.append(jax.core.ShapedArray(shape, dtype))
                zero_outs.append(np.zeros(shape, dtype))
        self.in_names = list(in_names)
        self.out_names = out_names
        self.out_avals = out_avals
        self.zero_outs = zero_outs
        all_names = in_names + out_names

        def _body(*args):
            outs = bass2jax._bass_exec_p.bind(
                *args,
                out_avals=tuple(out_avals),
                in_names=tuple(all_names),
                out_names=tuple(out_names),
                lowering_input_output_aliases=(),
                sim_require_finite=True,
                sim_require_nnan=True,
                nc=nc,
            )
            return tuple(outs)

        devices = jax.devices()[:n_cores]
        self.mesh = Mesh(np.asarray(devices), ("core",))
        n_args = len(all_names)
        self.sharded = jax.jit(
            shard_map(
                _body,
                mesh=self.mesh,
                in_specs=(PartitionSpec("core"),) * n_args,
                out_specs=(PartitionSpec("core"),) * len(out_names),
                check_rep=False,
            )
        )

    def concat_inputs(self, in_maps):
        cols = []
        for name in self.in_names:
            if name == "partition_id":
                cols.append(
                    np.arange(self.n_cores, dtype=np.uint32).reshape(
                        self.n_cores, 1
                    )
                )
            else:
                cols.append(
                    np.concatenate([np.asarray(m[name]) for m in in_maps], axis=0)
                )
        return cols

    def device_put(self, concat_in):
        """Place concatenated inputs (and zero output buffers) on the mesh."""
        from jax.sharding import NamedSharding, PartitionSpec

        sh = NamedSharding(self.mesh, PartitionSpec("core"))
        args = concat_in + [
            np.zeros((self.n_cores * z.shape[0], *z.shape[1:]), z.dtype)
            for z in self.zero_outs
        ]
        return [self.jax.device_put(a, sh) for a in args]

    def run_dev(self, dev_args):
        return self.sharded(*dev_args)

    def make_bench(self, n_reps: int):
        """Jitted fn executing the NEFF n_reps times serially on-device."""
        import jax
        from concourse import bass2jax
        from jax.experimental.shard_map import shard_map
        from jax.sharding import Mesh, PartitionSpec

        nc = self._nc
        out_avals = self.out_avals
        all_names = self.in_names + self.out_names
        out_names = self.out_names

        def _body(*args):
            outs = None
            for _ in range(n_reps):
                outs = bass2jax._bass_exec_p.bind(
                    *args,
                    out_avals=tuple(out_avals),
                    in_names=tuple(all_names),
                    out_names=tuple(out_names),
                    lowering_input_output_aliases=(),
                    sim_require_finite=True,
                    sim_require_nnan=True,
                    nc=nc,
                )
            return tuple(outs)

        n_args = len(all_names)
        return jax.jit(
            shard_map(
                _body,
                mesh=self.mesh,
                in_specs=(PartitionSpec("core"),) * n_args,
                out_specs=(PartitionSpec("core"),) * len(out_names),
                check_rep=False,
            )
        )

    def run(self, in_maps):
        dev_args = self.device_put(self.concat_inputs(in_maps))
        out_arrs = self.sharded(*dev_args)
        outs = []
        for c in range(self.n_cores):
            outs.append(
                {
                    name: np.asarray(out_arrs[i]).reshape(
                        self.n_cores, *self.out_avals[i].shape
                    )[c]
                    for i, name in enumerate(self.out_names)
                }
            )
        return outs


def _get_runner(causal: bool) -> Runner:
    key = ("runner", causal)
    if key not in _CACHE:
        _CACHE[key] = Runner(_get_nc(causal))
    return _CACHE[key]


def run_spmd(in_maps, causal: bool = True, **kw):
    nc = _get_nc(causal)
    return run_bass_kernel_spmd(nc, in_maps, core_ids=list(range(8)), **kw)


def kernel(x, mask, w_qkv, w_out):
    kind = _classify_mask(mask)
    if kind == "other":
        return _reference_numpy(x, mask, w_qkv, w_out)
    in_maps = make_in_maps(x, w_qkv, w_out)
    res = run_spmd(in_maps, causal=(kind == "causal"))
    out = np.zeros((B, S, D), np.float32)
    for c in range(8):
        out[c // 4] += res.results[c]["outp"]
    return out


if __name__ == "__main__":
    rng = np.random.default_rng(0)
    x = rng.standard_normal((B, S, D)).astype(np.float32)
    mask = np.tril(np.ones((S, S), np.int32)).reshape(1, 1, S, S)
    w_qkv = (rng.standard_normal((D, 3 * D)) * 0.02).astype(np.float32)
    w_out = (rng.standard_normal((D, D)) * 0.02).astype(np.float32)
    got = kernel(x, mask, w_qkv, w_out)
    print("kernel ran, out shape", got.shape)



# revision 7
# speedup vs baseline: 1.0777x; 1.0777x over previous
"""Multi-head attention (RoPE, causal) Trainium2 Bass kernel, v2.

Problem: nn_MultiHeadAttention_62431644615193
  x: [2, 2048, 1024] f32, mask: causal tril, w_qkv: [1024, 3072], w_out: [1024, 1024]

Sharding: 8 cores = batch(2) x head-groups(4 heads each). Each core emits a
bf16 partial [2048, 1024] (its heads through w_out rows); host sums 4
partials per batch in f32.

v2 changes vs baseline:
  - bf16 for qkT/phat/va/ao/w_out: exact-width diagonal blocks (no fp32r
    moving>=256 constraint), 2x DVE modes, half SBUF traffic.
  - ACT engine does ONLY exp (den reciprocal reads PSUM directly on DVE;
    psv->va and qk_raw copies moved to Pool).
  - startup DMAs spread across SP/ACT/DVE queues in consumption order.
  - proj/outproj PE work interleaved INTO the attention unit stream with
    deadline-based slots, so PE fills exp-wait bubbles and ACT stays the
    pacer nowhere.
  - PSUM: qk 2x[128,2,512] (4 banks) + pv 1x[65,2,512] (2) + shared aux
    2x[128,512] (2) = 8 banks exactly.
"""

import math

import numpy as np
import ml_dtypes

import concourse.bass as bass
import concourse.tile as tile
from concourse import bacc
import concourse.mybir as mybir
from concourse.bass_utils import run_bass_kernel_spmd

B, S, D = 2, 2048, 1024
H = 16
HD = D // H          # 64
HPC = H // 4         # 4 heads per core
ROPE_BASE = 10000.0

F32 = mybir.dt.float32
F32R = mybir.dt.float32r
BF16 = mybir.dt.bfloat16
AF = mybir.ActivationFunctionType

NT = S // 128        # 16 t-blocks
NI = 4               # 512-wide s-chunks

SECTIONS: list = []  # (start_instruction_id, label) in emit order


def build_nc(causal: bool = True):
    nc = bacc.Bacc("TRN2", target_bir_lowering=False, debug=False, num_devices=8)
    SECTIONS.clear()

    def mark(label):
        SECTIONS.append((nc.next_id(), label))

    xT = nc.dram_tensor("xT", [D, S], BF16, kind="ExternalInput")
    w_qk = nc.dram_tensor("w_qk", [D, 8 * HD], BF16, kind="ExternalInput")
    w_v = nc.dram_tensor("w_v", [D, 4 * HD], BF16, kind="ExternalInput")
    w_out = nc.dram_tensor("w_out", [4 * HD, D], BF16, kind="ExternalInput")
    cos2 = nc.dram_tensor("cos2", [128, S], BF16, kind="ExternalInput")
    sin2 = nc.dram_tensor("sin2", [128, S], BF16, kind="ExternalInput")
    rotP = nc.dram_tensor("rotP", [128, 128], BF16, kind="ExternalInput")
    mask01 = nc.dram_tensor("mask01", [128, 2 * 128], BF16, kind="ExternalInput")
    outp = nc.dram_tensor("outp", [S, D], BF16, kind="ExternalOutput")

    with tile.TileContext(nc) as tc:
        with (
            tc.tile_pool(name="const", bufs=1) as cpool,
            tc.tile_pool(name="qkT", bufs=1) as qkTpool,
            tc.tile_pool(name="va", bufs=1) as vapool,
            tc.tile_pool(name="xq", bufs=2) as xqpool,
            tc.tile_pool(name="qkraw", bufs=2) as qkrawpool,
            tc.tile_pool(name="ropescratch", bufs=2) as rspool,
            tc.tile_pool(name="phat", bufs=4) as phatpool,
            tc.tile_pool(name="norm", bufs=2) as normpool,
            tc.tile_pool(name="attn_out", bufs=4) as aopool,
            tc.tile_pool(name="outstage", bufs=3) as ostpool,
            tc.tile_pool(name="psqk", bufs=1, space="PSUM") as qkps,
            tc.tile_pool(name="pspv", bufs=1, space="PSUM") as pvps,
            tc.tile_pool(name="psaux", bufs=1, space="PSUM") as auxps,
        ):
            # ---------------- constants (consolidated tiles) ----------------
            w_qk_t = cpool.tile([128, 8, 8 * HD], BF16, name="wqkt", tag="wqkt")
            w_v_t = cpool.tile([128, 8, 4 * HD], BF16, name="wvt", tag="wvt")
            w_out_t = cpool.tile([128, 2, D], BF16, name="woutt", tag="woutt")
            rotP_t = cpool.tile([128, 128], BF16)
            cos_t = cpool.tile([128, S], BF16, name="cos_t", tag="cos_t")
            sin_t = cpool.tile([128, S], BF16, name="sin_t", tag="sin_t")
            mask01_t = cpool.tile([128, 2, 128], BF16)

            # --- startup DMA plan: batched transfers (per-DMA overhead is
            # ~0.9us, so few big DMAs beat many small ones), consumption-
            # ordered across SP (w_qk, xq1..3, half the out stores), ACT (xq0,
            # trig q0/q1, rotP, other half of stores), gpsimd-SWDGE (mask01,
            # w_v, trig q2/q3, w_out -- few gens, off the early Pool path).
            def dd_slab(dram, a, b, cols=None):
                """dram rows [128a, 128b) as [128, b-a, cols] slab."""
                sl = dram[128 * a : 128 * b, :] if cols is None else dram[128 * a : 128 * b, cols]
                return sl.rearrange("(dd p) s -> p dd s", p=128)

            xq_tiles: dict[int, object] = {}

            def xq_first(q):
                xq = xqpool.tile([128, 8, 512], BF16, tag="xq", name=f"xq{q}")
                xq_tiles[q] = xq
                return xq

            def issue_xq(q, eng, split=(4,)):
                xq = xq_first(q)
                s_sl = slice(512 * q, 512 * q + 512)
                lo = 0
                for n in (*split, 8):
                    if n > lo:
                        eng.dma_start(xq[:, lo:n, :], dd_slab(xT, lo, n, s_sl))
                    lo = n

            # The cost model serializes ALL transfers on one ~352GB/s pipe;
            # queues only parallelize the 0.6us configs. Transfer order ==
            # config-completion order, so alternate SP/ACT configs in the
            # exact consumption order and keep everything off gpsimd (whose
            # SWDGE gens would jump the queue) except tiny mask01.
            va_t = vapool.tile([128, NT, 4, HD + 1], BF16)
            nc.gpsimd.memset(va_t[:, :, :, HD : HD + 1], 1.0)
            nc.sync.dma_start(w_qk_t[:, 0:2, :], dd_slab(w_qk, 0, 2))
            nc.scalar.dma_start(
                xq_first(0)[:, 0:2, :], dd_slab(xT, 0, 2, slice(0, 512))
            )
            nc.sync.dma_start(w_qk_t[:, 2:4, :], dd_slab(w_qk, 2, 4))
            nc.scalar.dma_start(
                xq_tiles[0][:, 2:4, :], dd_slab(xT, 2, 4, slice(0, 512))
            )
            nc.sync.dma_start(w_qk_t[:, 4:8, :], dd_slab(w_qk, 4, 8))
            nc.scalar.dma_start(
                xq_tiles[0][:, 4:8, :], dd_slab(xT, 4, 8, slice(0, 512))
            )
            nc.scalar.dma_start(rotP_t[:], rotP[:])
            nc.scalar.dma_start(cos_t[:, 0:512], cos2[:, 0:512])
            nc.scalar.dma_start(sin_t[:, 0:512], sin2[:, 0:512])
            nc.sync.dma_start(w_v_t[:], dd_slab(w_v, 0, 8))
            nc.sync.dma_start(
                mask01_t[:], mask01[:].rearrange("p (b s) -> p b s", b=2)
            )
            issue_xq(1, nc.sync)
            nc.scalar.dma_start(cos_t[:, 512:1024], cos2[:, 512:1024])
            nc.scalar.dma_start(sin_t[:, 512:1024], sin2[:, 512:1024])
            nc.sync.dma_start(cos_t[:, 1024:2048], cos2[:, 1024:2048])
            nc.sync.dma_start(sin_t[:, 1024:2048], sin2[:, 1024:2048])
            nc.sync.dma_start(
                w_out_t[:], w_out[:].rearrange("(kk p) s -> p kk s", p=128)
            )

            w_out_f = w_out_t[:].rearrange("p a b -> p (a b)")
            # rotated qT/kT (bf16): [q_h0;q_h1], [k_h0;k_h1], [q_h2;q_h3], [k_h2;k_h3]
            qkT = [qkTpool.tile([128, S], BF16, name=f"qkT{i}", tag=f"qkT{i}") for i in range(4)]

            # ---------------- projection pieces ----------------
            def proj_group(q, mt):
                """qk-projection matmul group; returns qk_raw SBUF copy."""
                mark(f"pg{q}.{mt}")
                xq = xq_tiles[q]
                ps = auxps.tile([128, 512], F32, tag="aux", bufs=2, name="ps_g")
                for dd in range(8):
                    nc.tensor.matmul(
                        ps[:],
                        w_qk_t[:, dd, 128 * mt : 128 * mt + 128],
                        xq[:, dd, :],
                        start=(dd == 0),
                        stop=(dd == 7),
                    )
                qk_raw = qkrawpool.tile([128, 512], BF16, tag="qkraw", name="qk_raw")
                nc.scalar.copy(qk_raw[:], ps[:])
                return qk_raw

            def proj_rot(q, mt, qk_raw):
                """RoPE combine -> qkT[mt][:, quarter q] (bf16). rotate_half
                via a PE permutation matmul (neuronxcc forbids SB+SB operand
                base-partition mismatch, so no partition-offset DVE rotate);
                the elementwise combine is all-bf16 for the 2x DVE mode."""
                mark(f"rot{q}.{mt}")
                s_sl = slice(512 * q, 512 * q + 512)
                psr = auxps.tile([128, 512], F32, tag="aux", bufs=2, name="psr")
                nc.tensor.matmul(psr[:], rotP_t[:], qk_raw[:], start=True, stop=True)
                rotsin = rspool.tile([128, 512], BF16, tag="rs", name="rotsin")
                nc.vector.tensor_mul(rotsin[:], psr[:], sin_t[:, s_sl])
                qkcos = rspool.tile([128, 512], BF16, tag="qkcos", name="qkcos")
                nc.vector.tensor_mul(qkcos[:], qk_raw[:], cos_t[:, s_sl])
                nc.vector.tensor_add(qkT[mt][:, s_sl], qkcos[:], rotsin[:])

            def v_group(q, st):
                """v-projection for t-block j = 4q+st -> va (bf16)."""
                mark(f"vg{q}.{st}")
                j = 4 * q + st
                xq = xq_tiles[q]
                psv = auxps.tile([128, 4 * HD], F32, tag="aux", bufs=2, name="psv")
                for dd in range(8):
                    nc.tensor.matmul(
                        psv[:],
                        xq[:, dd, 128 * st : 128 * st + 128],
                        w_v_t[:, dd, :],
                        start=(dd == 0),
                        stop=(dd == 7),
                    )
                nc.vector.tensor_copy(
                    va_t[:, j, :, 0:HD], psv[:].rearrange("p (h c) -> p h c", h=4)
                )

            # ---------------- outproj ----------------
            def outproj_tile(i, st, ao_pair, tailmode=False):
                mark(f"op{i}.{st}")
                ssl = slice(512 * i + 128 * st, 512 * i + 128 * st + 128)
                sloc = slice(128 * st, 128 * st + 128)
                ostage = ostpool.tile([128, D], BF16, tag="ost", name="ostage")
                ps_t = (
                    qkps.tile([128, 2, 512], F32, tag="qk", bufs=2, name="ps_ot")
                    if tailmode and st >= 2 else None
                )
                for n2 in range(2):
                    ps_o = (
                        ps_t[:, n2, :] if ps_t is not None
                        else auxps.tile([128, 512], F32, tag="aux", bufs=2, name="ps_o")[:]
                    )
                    for kk in range(2):
                        nc.tensor.matmul(
                            ps_o,
                            ao_pair[kk][:, sloc],
                            w_out_f[:, D * kk + 512 * n2 : D * kk + 512 * n2 + 512],
                            start=(kk == 0),
                            stop=(kk == 1),
                        )
                    dst = ostage[:, 512 * n2 : 512 * n2 + 512]
                    if n2 == 1 and tailmode:
                        nc.scalar.copy(dst, ps_o)
                    else:
                        nc.vector.tensor_copy(dst, ps_o)
                    eng = nc.sync if tailmode or (st + n2) % 2 == 0 else nc.scalar
                    eng.dma_start(outp[ssl, 512 * n2 : 512 * n2 + 512], dst)

            # ---------------- attention ----------------
            ao_tiles: dict[tuple[int, int], object] = {}

            def make_attention(i, hp, interleave, tail=False):
                """Emit attention chunk i, head-pair hp. interleave = list of
                (slot, fn): fn fires after unit `slot` (-1 = before qk(0)).
                Causal: the pv-psum column subtile [128o, 128(o+1)) is final
                right after diagonal unit o, so the WAR-critical PSUM reads
                (recip + unnormalized copy-out) fire per-subtile inside the
                unit stream, and in tail mode normalize+outproj pipeline there
                too. Returns finalize() producing ao (no-op for tail)."""
                nblk = 4 * i + 4 if causal else NT
                qt = qkT[2 * hp]
                kt = qkT[2 * hp + 1]
                ps_pv = pvps.tile([65, 2, 512], F32, tag="pv", bufs=1, name="ps_pv")
                den_sb = normpool.tile([1, 2, 512], F32, tag="den", name="den_sb")
                rec1 = normpool.tile([1, 2, 512], F32, tag="rec1", name="rec1")
                un = ao = rec128 = ao_pair = None
                if not tail:
                    un = aopool.tile([64, 2, 512], F32, tag="unt", name=f"un_{i}_{hp}")
                else:
                    rec128 = normpool.tile([64, 2, 512], F32, tag="rec128", name="rec128")
                ao = aopool.tile([128, 512], BF16, tag="aot", name=f"ao_{i}_{hp}")
                ao_tiles[(i, hp)] = ao
                if tail:
                    ao_pair = [ao_tiles[(i, 0)], ao]

                def reg_of(j):
                    dvr = causal and 4 * i <= j
                    o = j - 4 * i if dvr else 0
                    return dvr, o, slice(128 * o, 512)

                def issue_qk(j):
                    mark(f"qk{i}.{hp}.{j}")
                    dvr, o, reg = reg_of(j)
                    ps_qk = qkps.tile([128, 2, 512], F32, tag="qk", bufs=2, name="ps_qk")
                    for sl2 in range(2):
                        hb = 64 * sl2
                        nc.tensor.matmul(
                            ps_qk[:, sl2, reg],
                            kt[hb : hb + 64, 128 * j : 128 * j + 128],
                            qt[hb : hb + 64, 512 * i + reg.start : 512 * i + 512],
                            start=True,
                            stop=True,
                        )
                    phat = phatpool.tile([128, 2, 512], BF16, tag="phat", name="phat")
                    if reg.start == 0:
                        nc.scalar.activation(
                            phat[:, :, :], ps_qk[:, :, :], AF.Exp,
                            scale=1.0 / math.sqrt(HD),
                        )
                    else:
                        ph_f = phat[:].rearrange("p b c -> p (b c)")
                        qk_f = ps_qk[:].rearrange("p b c -> p (b c)")
                        for sl2 in range(2):
                            lo = 512 * sl2 + reg.start
                            hi = 512 * sl2 + 512
                            nc.scalar.activation(
                                ph_f[:, lo:hi], qk_f[:, lo:hi], AF.Exp,
                                scale=1.0 / math.sqrt(HD),
                            )
                    if dvr:
                        strip = slice(128 * o, 128 * o + 128)
                        nc.vector.tensor_mul(
                            phat[:, :, strip], phat[:, :, strip], mask01_t[:]
                        )
                    return phat

                def sub_read(o):
                    """PSUM reads for finalized column subtile o (fires right
                    after diagonal unit o's pv)."""
                    mark(f"sr{i}.{hp}.{o}")
                    sub = slice(128 * o, 128 * o + 128)
                    nc.vector.tensor_copy(den_sb[:, 0, sub], ps_pv[HD : HD + 1, 0, sub])
                    nc.scalar.copy(den_sb[:, 1, sub], ps_pv[HD : HD + 1, 1, sub])
                    nc.vector.reciprocal_approx_fast(
                        rec1[:, :, sub], den_sb[:, :, sub]
                    )
                    if not tail:
                        nc.vector.tensor_copy(un[:, 0, sub], ps_pv[0:HD, 0, sub])
                        nc.vector.tensor_copy(un[:, 1, sub], ps_pv[0:HD, 1, sub])
                        return
                    # tail: normalize from PSUM + outproj, pipelined per sub
                    for sl2 in range(2):
                        nc.gpsimd.partition_broadcast(
                            rec128[:, sl2, sub], rec1[:, sl2, sub]
                        )
                        nc.vector.tensor_mul(
                            ao[64 * sl2 : 64 * sl2 + 64, sub],
                            ps_pv[0:HD, sl2, sub],
                            rec128[:, sl2, sub],
                        )
                    # defer the outproj matmuls one unit: emitted while their
                    # normalize chain is still running they'd clog the PE
                    # wait-queue and stall later (ready) units behind them
                    pending_op.append(o)

                def fire(slot):
                    for sl, fn in interleave:
                        if sl == slot:
                            fn()

                fire(-1)
                pending_op: list = []

                def flush_ops():
                    while pending_op:
                        outproj_tile(i, pending_op.pop(0), ao_pair, tailmode=True)
                phats = {0: issue_qk(0)}
                fire(-2)  # filler AFTER qk(0) is issued (exp chain started)
                for j in range(nblk):
                    if j + 1 < nblk:
                        phats[j + 1] = issue_qk(j + 1)
                    mark(f"pv{i}.{hp}.{j}")
                    _, _, reg = reg_of(j)
                    phat = phats.pop(j)
                    phat_f = phat[:].rearrange("p b c -> p (b c)")
                    for sl2 in range(2):
                        # skip_group_check: the causal structure finalizes
                        # column subtile [128o, 128(o+1)) right after diagonal
                        # unit o, and we read those PSUM columns before the
                        # bank's accumulation group formally stops (disjoint
                        # columns; physically race-free). rhs must be a flat
                        # single-free-dim AP: hardware miscomputes matmul rhs
                        # APs that combine a middle-dim index with a column
                        # offset (bf16).
                        nc.tensor.matmul(
                            ps_pv[:, sl2, reg],
                            va_t[:, j, 2 * hp + sl2, :],
                            phat_f[:, 512 * sl2 + reg.start : 512 * sl2 + 512],
                            start=(j == 0),
                            stop=(j == nblk - 1),
                            skip_group_check=True,
                        )
                    if causal and j >= 4 * i:
                        sub_read(j - 4 * i)
                    fire(j)
                    while len(pending_op) > 1:
                        outproj_tile(i, pending_op.pop(0), ao_pair, tailmode=True)
                flush_ops()
                if not causal:
                    nc.vector.tensor_copy(den_sb[:, 0, :], ps_pv[HD : HD + 1, 0, :])
                    nc.scalar.copy(den_sb[:, 1, :], ps_pv[HD : HD + 1, 1, :])
                    nc.vector.reciprocal_approx_fast(rec1[:], den_sb[:])
                    nc.vector.tensor_copy(un[:, 0, :], ps_pv[0:HD, 0, :])
                    nc.vector.tensor_copy(un[:, 1, :], ps_pv[0:HD, 1, :])

                if tail:
                    return lambda: None

                def finalize():
                    mark(f"fin{i}.{hp}")
                    rec = normpool.tile([64, 2, 512], F32, tag="rec128", name="rec128")
                    for sl2 in range(2):
                        nc.gpsimd.partition_broadcast(rec[:, sl2, :], rec1[:, sl2, :])
                        nc.vector.tensor_mul(
                            ao[64 * sl2 : 64 * sl2 + 64, :], un[:, sl2, :], rec[:, sl2, :]
                        )

                return finalize

            # ---------------- schedule ----------------
            if causal:
                # prelude: quarter 0 q-pair0/k-pair0 + all v; rots trail groups
                r0 = proj_group(0, 0)
                r1 = proj_group(0, 1)
                proj_rot(0, 0, r0)
                v_group(0, 0)
                proj_rot(0, 1, r1)
                v_group(0, 1)
                v_group(0, 2)
                v_group(0, 3)

                fins: dict = {}

                def hp0_items(i):
                    items = []
                    st_box = {}

                    def g(mt, key):
                        def f():
                            st_box[key] = proj_group(i, mt)
                        return f

                    def r(mt, key):
                        def f():
                            proj_rot(i, mt, st_box[key])
                        return f

                    if i > 0:
                        vs = {1: [2, 3, 4, 5], 2: [4, 6, 8, 9], 3: [6, 8, 10, 11]}[i]
                        items += [
                            (-2, g(1, "k")), (-2, lambda: v_group(i, 0)),
                            (0, fins[(i - 1, 1)]), (0, g(2, "q")),
                            (1, r(1, "k")), (2, r(2, "q")),
                        ]
                        items += [
                            (vs[st], (lambda s: lambda: v_group(i, s))(st))
                            for st in range(1, 4)
                        ]
                        if i < NI - 1:
                            items += [(6, g(3, "k2")), (7, r(3, "k2"))]
                    else:
                        items += [
                            (-2, g(2, "q")), (0, r(2, "q")),
                            (1, g(3, "k2")), (2, r(3, "k2")),
                        ]
                    return items

                def hp1_items(i):
                    items = []
                    if i + 2 < NI:
                        items.append((-2, lambda: issue_xq(i + 2, nc.sync)))
                    if i > 0:
                        ao_pair = [ao_tiles[(i - 1, 0)], ao_tiles[(i - 1, 1)]]
                        # last chunk: spread outproj(i-1) into the ACT-paced
                        # diagonal endgame instead of bunching it up front
                        slots = [-2, 8, 12, 14] if i == NI - 1 else [-2, -2, 2, 4]
                        for st in range(4):
                            items.append(
                                (slots[st], (lambda s: lambda: outproj_tile(i - 1, s, ao_pair))(st))
                            )
                    items.append((0, lambda: fins[(i, 0)]()))
                    if i == NI - 1:
                        st_box3 = {}

                        def g3():
                            st_box3["k"] = proj_group(i, 3)

                        def r3():
                            proj_rot(i, 3, st_box3["k"])

                        items += [(5, g3), (7, r3)]
                    if i < NI - 1:
                        st_box = {}

                        def g():
                            st_box["q"] = proj_group(i + 1, 0)

                        def r():
                            proj_rot(i + 1, 0, st_box["q"])

                        items += [(6 if i > 0 else -2, g), (7 if i > 0 else 1, r)]
                    return items

                for i in range(NI):
                    fins[(i, 0)] = make_attention(i, 0, hp0_items(i))
                    fins[(i, 1)] = make_attention(
                        i, 1, hp1_items(i), tail=(i == NI - 1)
                    )
            else:
                # non-causal: project everything first, then attention chunks
                # with outproj(i-1) interleaved.
                issue_xq(2, nc.sync)
                for q in range(NI):
                    if q == 2:
                        issue_xq(3, nc.sync)
                    rr = [proj_group(q, mt) for mt in range(4)]
                    for mt in range(4):
                        proj_rot(q, mt, rr[mt])
                    for st in range(4):
                        v_group(q, st)
                for i in range(NI):
                    items = []
                    if i > 0:
                        ao_pair = [ao_tiles[(i - 1, 0)], ao_tiles[(i - 1, 1)]]
                        for st in range(4):
                            items.append(
                                (st, (lambda s, p: lambda: outproj_tile(i - 1, s, p))(st, ao_pair))
                            )
                    make_attention(i, 0, items)()
                    make_attention(i, 1, [])()
                ao_pair = [ao_tiles[(NI - 1, 0)], ao_tiles[(NI - 1, 1)]]
                for st in range(4):
                    outproj_tile(NI - 1, st, ao_pair)

    nc.compile()
    return nc


# --------------------------------------------------------------------------
# host-side: constants, sharding, assembly
# --------------------------------------------------------------------------

def _rope_tables():
    inv_freq = 1.0 / (ROPE_BASE ** (np.arange(0, HD, 2, dtype=np.float32) / HD))
    positions = np.arange(S, dtype=np.float32)
    freqs = np.outer(positions, inv_freq).astype(np.float32)     # [S, 32]
    emb = np.concatenate((freqs, freqs), axis=-1)                # [S, 64]
    cosT = np.cos(emb).T.astype(np.float32)                      # [64, S]
    sinT = np.sin(emb).T.astype(np.float32)
    cos2 = np.vstack([cosT, cosT]).astype(ml_dtypes.bfloat16)    # [128, S]
    sin2 = np.vstack([sinT, sinT]).astype(ml_dtypes.bfloat16)
    return cos2, sin2


def _rot_lhsT():
    # rotate_half (interleaved): rot[2i] = -x[2i+1], rot[2i+1] = x[2i]
    P = np.zeros((128, 128), np.float32)
    for base in (0, 64):
        for i2 in range(HD // 2):
            P[base + 2 * i2, base + 2 * i2 + 1] = -1.0
            P[base + 2 * i2 + 1, base + 2 * i2] = 1.0
    return np.ascontiguousarray(P.T).astype(ml_dtypes.bfloat16)


_CACHE: dict = {}


def _get_nc(causal: bool):
    key = ("nc", causal)
    if key not in _CACHE:
        _CACHE[key] = build_nc(causal)
    return _CACHE[key]


def _classify_mask(mask: np.ndarray) -> str:
    m = np.asarray(mask).reshape(S, S)
    if np.array_equal(m != 0, np.tril(np.ones((S, S), bool))):
        return "causal"
    if np.all(m != 0):
        return "full"
    return "other"


def make_in_maps(x, w_qkv, w_out):
    """Build the 8 per-core input dicts."""
    cos2, sin2 = _rope_tables()
    rotP = _rot_lhsT()
    m01 = (np.arange(128)[:, None] <= np.arange(128)[None, :]).astype(np.float32)
    mask01 = np.ascontiguousarray(
        np.stack([m01, m01], axis=1).reshape(128, 256)
    ).astype(ml_dtypes.bfloat16)

    w3 = np.asarray(w_qkv).reshape(D, 3, H, HD)   # [D, {q,k,v}, H, hd]
    wo = np.asarray(w_out)                        # [D, D]; rows indexed [h, hd]
    xT = [
        np.ascontiguousarray(np.asarray(x)[b].T).astype(ml_dtypes.bfloat16)
        for b in range(B)
    ]  # [D, S]

    in_maps = []
    for c in range(8):
        b, hg = divmod(c, 4)
        hs = [4 * hg + i for i in range(HPC)]
        # w_qk cols: [q_h0, q_h1, k_h0, k_h1, q_h2, q_h3, k_h2, k_h3],
        # each head block column-permuted to the [evens; odds] layout
        wqk_cols = []
        for pair in range(2):
            for t in range(2):  # 0 = q, 1 = k
                for hh in (hs[2 * pair], hs[2 * pair + 1]):
                    wqk_cols.append(w3[:, t, hh, :])
        w_qk_c = np.ascontiguousarray(np.concatenate(wqk_cols, axis=1)).astype(
            ml_dtypes.bfloat16
        )  # [D, 512]
        w_v_c = np.ascontiguousarray(
            np.concatenate([w3[:, 2, hh, :] for hh in hs], axis=1)
        ).astype(ml_dtypes.bfloat16)  # [D, 256]
        w_out_c = np.ascontiguousarray(
            np.concatenate([wo[HD * hh : HD * hh + HD, :] for hh in hs], axis=0)
        ).astype(ml_dtypes.bfloat16)  # [256, D]
        in_maps.append(
            {
                "xT": xT[b],
                "w_qk": w_qk_c,
                "w_v": w_v_c,
                "w_out": w_out_c,
                "cos2": cos2,
                "sin2": sin2,
                "rotP": rotP,
                "mask01": mask01,
            }
        )
    return in_maps


def _reference_numpy(x, mask, w_qkv, w_out):
    """Exact fallback for non-causal, non-full masks (slow, host-side)."""
    x = np.asarray(x, np.float32)
    qkv = (x @ w_qkv).reshape(B, S, 3, H, HD)
    qkv = np.transpose(qkv, (2, 0, 3, 1, 4))
    q, k, v = qkv[0], qkv[1], qkv[2]
    inv_freq = 1.0 / (ROPE_BASE ** (np.arange(0, HD, 2, dtype=np.float32) / HD))
    freqs = np.outer(np.arange(S, dtype=np.float32), inv_freq)
    emb = np.concatenate((freqs, freqs), axis=-1)
    cos = np.cos(emb)[None, None]
    sin = np.sin(emb)[None, None]

    def rot(t):
        t1 = t[..., ::2]
        t2 = t[..., 1::2]
        return np.stack((-t2, t1), axis=-1).reshape(t.shape)

    q = q * cos + rot(q) * sin
    k = k * cos + rot(k) * sin
    attn = np.einsum("bhsd,bhtd->bhst", q, k) / math.sqrt(HD)
    m = np.asarray(mask).reshape(1, 1, S, S)
    attn = np.where(m == 0, -np.inf, attn)
    attn = attn - attn.max(-1, keepdims=True)
    np.exp(attn, out=attn)
    attn /= attn.sum(-1, keepdims=True)
    out = np.einsum("bhst,bhtd->bhsd", attn, v)
    out = np.transpose(out, (0, 2, 1, 3)).reshape(B, S, D)
    return (out @ w_out).astype(np.float32)


class Runner:
    """Cached jitted SPMD runner (mirrors bass2jax.run_bass_via_pjrt)."""

    def __init__(self, nc, n_cores: int = 8):
        import jax
        import concourse.mybir as _mybir
        from concourse import bass2jax
        from jax.experimental.shard_map import shard_map
        from jax.sharding import Mesh, PartitionSpec

        bass2jax.install_neuronx_cc_hook()
        self.jax = jax
        self.n_cores = n_cores
        self._nc = nc
        in_names, out_names, out_avals, zero_outs = [], [], [], []
        for alloc in nc.m.functions[0].allocations:
            if not isinstance(alloc, _mybir.MemoryLocationSet):
                continue
            name = alloc.memorylocations[0].name
            if alloc.kind == "ExternalInput":
                in_names.append(name)
            elif alloc.kind == "ExternalOutput":
                out_names.append(name)
                shape = tuple(alloc.tensor_shape)
                dtype = _mybir.dt.np(alloc.dtype)
                out_avals.append(jax.core.ShapedArray(shape, dtype))
                zero_outs.append(np.zeros(shape, dtype))
        self.in_names = list(in_names)
        self.out_names = out_names
        self.out_avals = out_avals
        self.zero_outs = zero_outs
        all_names = in_names + out_names

        def _body(*args):
            outs = bass2jax._bass_exec_p.bind(
                *args,
                out_avals=tuple(out_avals),
                in_names=tuple(all_names),
                out_names=tuple(out_names),
                lowering_input_output_aliases=(),
                sim_require_finite=True,
                sim_require_nnan=True,
                nc=nc,
            )
            return tuple(outs)

        devices = jax.devices()[:n_cores]
        self.mesh = Mesh(np.asarray(devices), ("core",))
        n_args = len(all_names)
        self.sharded = jax.jit(
            shard_map(
                _body,
                mesh=self.mesh,
                in_specs=(PartitionSpec("core"),) * n_args,
                out_specs=(PartitionSpec("core"),) * len(out_names),
                check_rep=False,
            )
        )

    def concat_inputs(self, in_maps):
        cols = []
        for name in self.in_names:
            if name == "partition_id":
                cols.append(
                    np.arange(self.n_cores, dtype=np.uint32).reshape(self.n_cores, 1)
                )
            else:
                cols.append(
                    np.concatenate([np.asarray(m[name]) for m in in_maps], axis=0)
                )
        return cols

    def device_put(self, concat_in):
        from jax.sharding import NamedSharding, PartitionSpec

        sh = NamedSharding(self.mesh, PartitionSpec("core"))
        args = concat_in + [
            np.zeros((self.n_cores * z.shape[0], *z.shape[1:]), z.dtype)
            for z in self.zero_outs
        ]
        return [self.jax.device_put(a, sh) for a in args]

    def run_dev(self, dev_args):
        return self.sharded(*dev_args)

    def make_bench(self, n_reps: int):
        import jax
        from concourse import bass2jax
        from jax.experimental.shard_map import shard_map
        from jax.sharding import Mesh, PartitionSpec

        nc = self._nc
        out_avals = self.out_avals
        all_names = self.in_names + self.out_names
        out_names = self.out_names

        def _body(*args):
            outs = None
            for _ in range(n_reps):
                outs = bass2jax._bass_exec_p.bind(
                    *args,
                    out_avals=tuple(out_avals),
                    in_names=tuple(all_names),
                    out_names=tuple(out_names),
                    lowering_input_output_aliases=(),
                    sim_require_finite=True,
                    sim_require_nnan=True,
                    nc=nc,
                )
            return tuple(outs)

        n_args = len(all_names)
        return jax.jit(
            shard_map(
                _body,
                mesh=self.mesh,
                in_specs=(PartitionSpec("core"),) * n_args,
                out_specs=(PartitionSpec("core"),) * len(out_names),
                check_rep=False,
            )
        )

    def run(self, in_maps):
        dev_args = self.device_put(self.concat_inputs(in_maps))
        out_arrs = self.sharded(*dev_args)
        outs = []
        for c in range(self.n_cores):
            outs.append(
                {
                    name: np.asarray(out_arrs[i]).reshape(
                        self.n_cores, *self.out_avals[i].shape
                    )[c]
                    for i, name in enumerate(self.out_names)
                }
            )
        return outs


def _get_runner(causal: bool) -> Runner:
    key = ("runner", causal)
    if key not in _CACHE:
        _CACHE[key] = Runner(_get_nc(causal))
    return _CACHE[key]


def run_spmd(in_maps, causal: bool = True, **kw):
    nc = _get_nc(causal)
    return run_bass_kernel_spmd(nc, in_maps, core_ids=list(range(8)), **kw)


def kernel(x, mask, w_qkv, w_out):
    kind = _classify_mask(mask)
    if kind == "other":
        return _reference_numpy(x, mask, w_qkv, w_out)
    in_maps = make_in_maps(x, w_qkv, w_out)
    res = run_spmd(in_maps, causal=(kind == "causal"))
    out = np.zeros((B, S, D), np.float32)
    for c in range(8):
        out[c // 4] += np.asarray(res.results[c]["outp"]).astype(np.float32)
    return out


if __name__ == "__main__":
    rng = np.random.default_rng(0)
    x = rng.standard_normal((B, S, D)).astype(np.float32)
    mask = np.tril(np.ones((S, S), np.int32)).reshape(1, 1, S, S)
    w_qkv = (rng.standard_normal((D, 3 * D)) * 0.02).astype(np.float32)
    w_out = (rng.standard_normal((D, D)) * 0.02).astype(np.float32)
    got = kernel(x, mask, w_qkv, w_out)
    print("kernel ran, out shape", got.shape)


# revision 8
# speedup vs baseline: 1.0794x; 1.0016x over previous
"""Multi-head attention (RoPE, causal) Trainium2 Bass kernel, v2.

Problem: nn_MultiHeadAttention_62431644615193
  x: [2, 2048, 1024] f32, mask: causal tril, w_qkv: [1024, 3072], w_out: [1024, 1024]

Sharding: 8 cores = batch(2) x head-groups(4 heads each). Each core emits a
bf16 partial [2048, 1024] (its heads through w_out rows); host sums 4
partials per batch in f32.

v2 changes vs baseline:
  - bf16 for qkT/phat/va/ao/w_out: exact-width diagonal blocks (no fp32r
    moving>=256 constraint), 2x DVE modes, half SBUF traffic.
  - ACT engine does ONLY exp (den reciprocal reads PSUM directly on DVE;
    psv->va and qk_raw copies moved to Pool).
  - startup DMAs spread across SP/ACT/DVE queues in consumption order.
  - proj/outproj PE work interleaved INTO the attention unit stream with
    deadline-based slots, so PE fills exp-wait bubbles and ACT stays the
    pacer nowhere.
  - PSUM: qk 2x[128,2,512] (4 banks) + pv 1x[65,2,512] (2) + shared aux
    2x[128,512] (2) = 8 banks exactly.
"""

import math

import numpy as np
import ml_dtypes

import concourse.bass as bass
import concourse.tile as tile
from concourse import bacc
import concourse.mybir as mybir
from concourse.bass_utils import run_bass_kernel_spmd

B, S, D = 2, 2048, 1024
H = 16
HD = D // H          # 64
HPC = H // 4         # 4 heads per core
ROPE_BASE = 10000.0

F32 = mybir.dt.float32
F32R = mybir.dt.float32r
BF16 = mybir.dt.bfloat16
AF = mybir.ActivationFunctionType

NT = S // 128        # 16 t-blocks
NI = 4               # 512-wide s-chunks

SECTIONS: list = []  # (start_instruction_id, label) in emit order


def build_nc(causal: bool = True):
    nc = bacc.Bacc("TRN2", target_bir_lowering=False, debug=False, num_devices=8)
    SECTIONS.clear()

    def mark(label):
        SECTIONS.append((nc.next_id(), label))

    xT = nc.dram_tensor("xT", [D, S], BF16, kind="ExternalInput")
    w_qk = nc.dram_tensor("w_qk", [D, 8 * HD], BF16, kind="ExternalInput")
    w_v = nc.dram_tensor("w_v", [D, 4 * HD], BF16, kind="ExternalInput")
    w_out = nc.dram_tensor("w_out", [4 * HD, D], BF16, kind="ExternalInput")
    cos2 = nc.dram_tensor("cos2", [128, S], BF16, kind="ExternalInput")
    sin2 = nc.dram_tensor("sin2", [128, S], BF16, kind="ExternalInput")
    rotP = nc.dram_tensor("rotP", [128, 128], BF16, kind="ExternalInput")
    mask01 = nc.dram_tensor("mask01", [128, 2 * 128], BF16, kind="ExternalInput")
    outp = nc.dram_tensor("outp", [S, D], BF16, kind="ExternalOutput")

    with tile.TileContext(nc) as tc:
        with (
            tc.tile_pool(name="const", bufs=1) as cpool,
            tc.tile_pool(name="qkT", bufs=1) as qkTpool,
            tc.tile_pool(name="va", bufs=1) as vapool,
            tc.tile_pool(name="xq", bufs=2) as xqpool,
            tc.tile_pool(name="qkraw", bufs=2) as qkrawpool,
            tc.tile_pool(name="ropescratch", bufs=2) as rspool,
            tc.tile_pool(name="phat", bufs=4) as phatpool,
            tc.tile_pool(name="norm", bufs=2) as normpool,
            tc.tile_pool(name="attn_out", bufs=4) as aopool,
            tc.tile_pool(name="outstage", bufs=3) as ostpool,
            tc.tile_pool(name="psqk", bufs=1, space="PSUM") as qkps,
            tc.tile_pool(name="pspv", bufs=1, space="PSUM") as pvps,
            tc.tile_pool(name="psaux", bufs=1, space="PSUM") as auxps,
        ):
            # ---------------- constants (consolidated tiles) ----------------
            w_qk_t = cpool.tile([128, 8, 8 * HD], BF16, name="wqkt", tag="wqkt")
            w_v_t = cpool.tile([128, 8, 4 * HD], BF16, name="wvt", tag="wvt")
            w_out_t = cpool.tile([128, 2, D], BF16, name="woutt", tag="woutt")
            rotP_t = cpool.tile([128, 128], BF16)
            cos_t = cpool.tile([128, S], BF16, name="cos_t", tag="cos_t")
            sin_t = cpool.tile([128, S], BF16, name="sin_t", tag="sin_t")
            mask01_t = cpool.tile([128, 2, 128], BF16)

            # --- startup DMA plan: batched transfers (per-DMA overhead is
            # ~0.9us, so few big DMAs beat many small ones), consumption-
            # ordered across SP (w_qk, xq1..3, half the out stores), ACT (xq0,
            # trig q0/q1, rotP, other half of stores), gpsimd-SWDGE (mask01,
            # w_v, trig q2/q3, w_out -- few gens, off the early Pool path).
            def dd_slab(dram, a, b, cols=None):
                """dram rows [128a, 128b) as [128, b-a, cols] slab."""
                sl = dram[128 * a : 128 * b, :] if cols is None else dram[128 * a : 128 * b, cols]
                return sl.rearrange("(dd p) s -> p dd s", p=128)

            xq_tiles: dict[int, object] = {}

            def xq_first(q):
                xq = xqpool.tile([128, 8, 512], BF16, tag="xq", name=f"xq{q}")
                xq_tiles[q] = xq
                return xq

            def issue_xq(q, eng, split=(4,)):
                xq = xq_first(q)
                s_sl = slice(512 * q, 512 * q + 512)
                lo = 0
                for n in (*split, 8):
                    if n > lo:
                        eng.dma_start(xq[:, lo:n, :], dd_slab(xT, lo, n, s_sl))
                    lo = n

            # The cost model serializes ALL transfers on one ~352GB/s pipe;
            # queues only parallelize the 0.6us configs. Transfer order ==
            # config-completion order, so alternate SP/ACT configs in the
            # exact consumption order and keep everything off gpsimd (whose
            # SWDGE gens would jump the queue) except tiny mask01.
            va_t = vapool.tile([128, NT, 4, HD + 1], BF16)
            nc.gpsimd.memset(va_t[:, :, :, HD : HD + 1], 1.0)
            nc.sync.dma_start(w_qk_t[:, 0:2, :], dd_slab(w_qk, 0, 2))
            nc.scalar.dma_start(
                xq_first(0)[:, 0:2, :], dd_slab(xT, 0, 2, slice(0, 512))
            )
            nc.sync.dma_start(w_qk_t[:, 2:4, :], dd_slab(w_qk, 2, 4))
            nc.scalar.dma_start(
                xq_tiles[0][:, 2:4, :], dd_slab(xT, 2, 4, slice(0, 512))
            )
            nc.sync.dma_start(w_qk_t[:, 4:8, :], dd_slab(w_qk, 4, 8))
            nc.scalar.dma_start(
                xq_tiles[0][:, 4:8, :], dd_slab(xT, 4, 8, slice(0, 512))
            )
            nc.scalar.dma_start(rotP_t[:], rotP[:])
            nc.scalar.dma_start(cos_t[:, 0:512], cos2[:, 0:512])
            nc.scalar.dma_start(sin_t[:, 0:512], sin2[:, 0:512])
            nc.sync.dma_start(w_v_t[:], dd_slab(w_v, 0, 8))
            nc.sync.dma_start(
                mask01_t[:], mask01[:].rearrange("p (b s) -> p b s", b=2)
            )
            issue_xq(1, nc.sync)
            nc.scalar.dma_start(cos_t[:, 512:1024], cos2[:, 512:1024])
            nc.scalar.dma_start(sin_t[:, 512:1024], sin2[:, 512:1024])
            nc.sync.dma_start(cos_t[:, 1024:2048], cos2[:, 1024:2048])
            nc.sync.dma_start(sin_t[:, 1024:2048], sin2[:, 1024:2048])
            nc.sync.dma_start(
                w_out_t[:], w_out[:].rearrange("(kk p) s -> p kk s", p=128)
            )

            w_out_f = w_out_t[:].rearrange("p a b -> p (a b)")
            # rotated qT/kT (bf16): [q_h0;q_h1], [k_h0;k_h1], [q_h2;q_h3], [k_h2;k_h3]
            qkT = [qkTpool.tile([128, S], BF16, name=f"qkT{i}", tag=f"qkT{i}") for i in range(4)]

            # ---------------- projection pieces ----------------
            def proj_group(q, mt):
                """qk-projection matmul group; returns qk_raw SBUF copy."""
                mark(f"pg{q}.{mt}")
                xq = xq_tiles[q]
                ps = auxps.tile([128, 512], F32, tag="aux", bufs=2, name="ps_g")
                for dd in range(8):
                    nc.tensor.matmul(
                        ps[:],
                        w_qk_t[:, dd, 128 * mt : 128 * mt + 128],
                        xq[:, dd, :],
                        start=(dd == 0),
                        stop=(dd == 7),
                    )
                qk_raw = qkrawpool.tile([128, 512], BF16, tag="qkraw", name="qk_raw")
                nc.scalar.copy(qk_raw[:], ps[:])
                return qk_raw

            def proj_rot(q, mt, qk_raw):
                """RoPE combine -> qkT[mt][:, quarter q] (bf16). rotate_half
                via a PE permutation matmul (neuronxcc forbids SB+SB operand
                base-partition mismatch, so no partition-offset DVE rotate);
                the elementwise combine is all-bf16 for the 2x DVE mode."""
                mark(f"rot{q}.{mt}")
                s_sl = slice(512 * q, 512 * q + 512)
                psr = auxps.tile([128, 512], F32, tag="aux", bufs=2, name="psr")
                nc.tensor.matmul(psr[:], rotP_t[:], qk_raw[:], start=True, stop=True)
                rotsin = rspool.tile([128, 512], BF16, tag="rs", name="rotsin")
                nc.vector.tensor_mul(rotsin[:], psr[:], sin_t[:, s_sl])
                qkcos = rspool.tile([128, 512], BF16, tag="qkcos", name="qkcos")
                nc.vector.tensor_mul(qkcos[:], qk_raw[:], cos_t[:, s_sl])
                nc.vector.tensor_add(qkT[mt][:, s_sl], qkcos[:], rotsin[:])

            def v_group(q, st):
                """v-projection for t-block j = 4q+st -> va (bf16)."""
                mark(f"vg{q}.{st}")
                j = 4 * q + st
                xq = xq_tiles[q]
                psv = auxps.tile([128, 4 * HD], F32, tag="aux", bufs=2, name="psv")
                for dd in range(8):
                    nc.tensor.matmul(
                        psv[:],
                        xq[:, dd, 128 * st : 128 * st + 128],
                        w_v_t[:, dd, :],
                        start=(dd == 0),
                        stop=(dd == 7),
                    )
                nc.vector.tensor_copy(
                    va_t[:, j, :, 0:HD], psv[:].rearrange("p (h c) -> p h c", h=4)
                )

            # ---------------- outproj ----------------
            def outproj_tile(i, st, ao_pair, tailmode=False):
                mark(f"op{i}.{st}")
                ssl = slice(512 * i + 128 * st, 512 * i + 128 * st + 128)
                sloc = slice(128 * st, 128 * st + 128)
                ostage = ostpool.tile([128, D], BF16, tag="ost", name="ostage")
                ps_t = (
                    qkps.tile([128, 2, 512], F32, tag="qk", bufs=2, name="ps_ot")
                    if tailmode and st >= 2 else None
                )
                for n2 in range(2):
                    ps_o = (
                        ps_t[:, n2, :] if ps_t is not None
                        else auxps.tile([128, 512], F32, tag="aux", bufs=2, name="ps_o")[:]
                    )
                    for kk in range(2):
                        nc.tensor.matmul(
                            ps_o,
                            ao_pair[kk][:, sloc],
                            w_out_f[:, D * kk + 512 * n2 : D * kk + 512 * n2 + 512],
                            start=(kk == 0),
                            stop=(kk == 1),
                        )
                    dst = ostage[:, 512 * n2 : 512 * n2 + 512]
                    if n2 == 1 and tailmode:
                        nc.scalar.copy(dst, ps_o)
                    else:
                        nc.vector.tensor_copy(dst, ps_o)
                    eng = nc.sync if tailmode or (st + n2) % 2 == 0 else nc.scalar
                    eng.dma_start(outp[ssl, 512 * n2 : 512 * n2 + 512], dst)

            # ---------------- attention ----------------
            ao_tiles: dict[tuple[int, int], object] = {}

            def make_attention(i, hp, interleave, tail=False):
                """Emit attention chunk i, head-pair hp. interleave = list of
                (slot, fn): fn fires after unit `slot` (-1 = before qk(0)).
                Causal: the pv-psum column subtile [128o, 128(o+1)) is final
                right after diagonal unit o, so the WAR-critical PSUM reads
                (recip + unnormalized copy-out) fire per-subtile inside the
                unit stream, and in tail mode normalize+outproj pipeline there
                too. Returns finalize() producing ao (no-op for tail)."""
                nblk = 4 * i + 4 if causal else NT
                qt = qkT[2 * hp]
                kt = qkT[2 * hp + 1]
                ps_pv = pvps.tile([65, 2, 512], F32, tag="pv", bufs=1, name="ps_pv")
                den_sb = normpool.tile([1, 2, 512], F32, tag="den", name="den_sb")
                rec1 = normpool.tile([1, 2, 512], F32, tag="rec1", name="rec1")
                un = ao = rec128 = ao_pair = None
                if not tail:
                    un = aopool.tile([64, 2, 512], F32, tag="unt", name=f"un_{i}_{hp}")
                else:
                    rec128 = normpool.tile([64, 2, 512], F32, tag="rec128", name="rec128")
                ao = aopool.tile([128, 512], BF16, tag="aot", name=f"ao_{i}_{hp}")
                ao_tiles[(i, hp)] = ao
                if tail:
                    ao_pair = [ao_tiles[(i, 0)], ao]

                def reg_of(j):
                    dvr = causal and 4 * i <= j
                    o = j - 4 * i if dvr else 0
                    return dvr, o, slice(128 * o, 512)

                def issue_qk(j):
                    """Diagonal blocks write their w = 512-128o valid columns
                    LEFT-SHIFTED to offset 0 in ps_qk/phat: zero-offset
                    multi-dim APs are the hardware-safe form for the exp
                    (middle-dim index + column offset corrupts), and the
                    causal strip is then always phat[:, :, 0:128]."""
                    mark(f"qk{i}.{hp}.{j}")
                    dvr, o, reg = reg_of(j)
                    w = 512 - reg.start
                    ps_qk = qkps.tile([128, 2, 512], F32, tag="qk", bufs=2, name="ps_qk")
                    for sl2 in range(2):
                        hb = 64 * sl2
                        nc.tensor.matmul(
                            ps_qk[:, sl2, 0:w],
                            kt[hb : hb + 64, 128 * j : 128 * j + 128],
                            qt[hb : hb + 64, 512 * i + reg.start : 512 * i + 512],
                            start=True,
                            stop=True,
                        )
                    phat = phatpool.tile([128, 2, 512], BF16, tag="phat", name="phat")
                    nc.scalar.activation(
                        phat[:, :, 0:w], ps_qk[:, :, 0:w], AF.Exp,
                        scale=1.0 / math.sqrt(HD),
                    )
                    if dvr:
                        nc.vector.tensor_mul(
                            phat[:, :, 0:128], phat[:, :, 0:128], mask01_t[:]
                        )
                    return phat

                def sub_read(o):
                    """PSUM reads for finalized column subtile o (fires right
                    after diagonal unit o's pv)."""
                    mark(f"sr{i}.{hp}.{o}")
                    sub = slice(128 * o, 128 * o + 128)
                    nc.vector.tensor_copy(den_sb[:, 0, sub], ps_pv[HD : HD + 1, 0, sub])
                    nc.scalar.copy(den_sb[:, 1, sub], ps_pv[HD : HD + 1, 1, sub])
                    nc.vector.reciprocal_approx_fast(
                        rec1[:, :, sub], den_sb[:, :, sub]
                    )
                    if not tail:
                        nc.vector.tensor_copy(un[:, 0, sub], ps_pv[0:HD, 0, sub])
                        nc.vector.tensor_copy(un[:, 1, sub], ps_pv[0:HD, 1, sub])
                        return
                    # tail: normalize from PSUM + outproj, pipelined per sub
                    for sl2 in range(2):
                        nc.gpsimd.partition_broadcast(
                            rec128[:, sl2, sub], rec1[:, sl2, sub]
                        )
                        nc.vector.tensor_mul(
                            ao[64 * sl2 : 64 * sl2 + 64, sub],
                            ps_pv[0:HD, sl2, sub],
                            rec128[:, sl2, sub],
                        )
                    # defer the outproj matmuls one unit: emitted while their
                    # normalize chain is still running they'd clog the PE
                    # wait-queue and stall later (ready) units behind them
                    pending_op.append(o)

                def fire(slot):
                    for sl, fn in interleave:
                        if sl == slot:
                            fn()

                fire(-1)
                pending_op: list = []

                def flush_ops():
                    while pending_op:
                        outproj_tile(i, pending_op.pop(0), ao_pair, tailmode=True)
                phats = {0: issue_qk(0)}
                fire(-2)  # filler AFTER qk(0) is issued (exp chain started)
                for j in range(nblk):
                    if j + 1 < nblk:
                        phats[j + 1] = issue_qk(j + 1)
                    mark(f"pv{i}.{hp}.{j}")
                    _, _, reg = reg_of(j)
                    phat = phats.pop(j)
                    phat_f = phat[:].rearrange("p b c -> p (b c)")
                    w = 512 - reg.start
                    for sl2 in range(2):
                        # skip_group_check: the causal structure finalizes
                        # column subtile [128o, 128(o+1)) right after diagonal
                        # unit o, and we read those PSUM columns before the
                        # bank's accumulation group formally stops (disjoint
                        # columns; physically race-free). rhs is a flat
                        # single-free-dim AP over the left-shifted phat
                        # (hardware-safe form).
                        nc.tensor.matmul(
                            ps_pv[:, sl2, reg],
                            va_t[:, j, 2 * hp + sl2, :],
                            phat_f[:, 512 * sl2 : 512 * sl2 + w],
                            start=(j == 0),
                            stop=(j == nblk - 1),
                            skip_group_check=True,
                        )
                    if causal and j >= 4 * i:
                        sub_read(j - 4 * i)
                    fire(j)
                    while len(pending_op) > 1:
                        outproj_tile(i, pending_op.pop(0), ao_pair, tailmode=True)
                flush_ops()
                if not causal:
                    nc.vector.tensor_copy(den_sb[:, 0, :], ps_pv[HD : HD + 1, 0, :])
                    nc.scalar.copy(den_sb[:, 1, :], ps_pv[HD : HD + 1, 1, :])
                    nc.vector.reciprocal_approx_fast(rec1[:], den_sb[:])
                    nc.vector.tensor_copy(un[:, 0, :], ps_pv[0:HD, 0, :])
                    nc.vector.tensor_copy(un[:, 1, :], ps_pv[0:HD, 1, :])

                if tail:
                    return lambda: None

                def finalize():
                    mark(f"fin{i}.{hp}")
                    rec = normpool.tile([64, 2, 512], F32, tag="rec128", name="rec128")
                    for sl2 in range(2):
                        nc.gpsimd.partition_broadcast(rec[:, sl2, :], rec1[:, sl2, :])
                        nc.vector.tensor_mul(
                            ao[64 * sl2 : 64 * sl2 + 64, :], un[:, sl2, :], rec[:, sl2, :]
                        )

                return finalize

            # ---------------- schedule ----------------
            if causal:
                # prelude: quarter 0 q-pair0/k-pair0 + all v; rots trail groups
                r0 = proj_group(0, 0)
                r1 = proj_group(0, 1)
                proj_rot(0, 0, r0)
                v_group(0, 0)
                proj_rot(0, 1, r1)
                v_group(0, 1)
                v_group(0, 2)
                v_group(0, 3)

                fins: dict = {}

                def hp0_items(i):
                    items = []
                    st_box = {}

                    def g(mt, key):
                        def f():
                            st_box[key] = proj_group(i, mt)
                        return f

                    def r(mt, key):
                        def f():
                            proj_rot(i, mt, st_box[key])
                        return f

                    if i > 0:
                        vs = {1: [2, 3, 4, 5], 2: [4, 6, 8, 9], 3: [6, 8, 10, 11]}[i]
                        items += [
                            (-2, g(1, "k")), (-2, lambda: v_group(i, 0)),
                            (0, fins[(i - 1, 1)]), (0, g(2, "q")),
                            (1, r(1, "k")), (2, r(2, "q")),
                        ]
                        items += [
                            (vs[st], (lambda s: lambda: v_group(i, s))(st))
                            for st in range(1, 4)
                        ]
                        if i < NI - 1:
                            items += [(6, g(3, "k2")), (7, r(3, "k2"))]
                    else:
                        items += [
                            (-2, g(2, "q")), (0, r(2, "q")),
                            (1, g(3, "k2")), (2, r(3, "k2")),
                        ]
                    return items

                def hp1_items(i):
                    items = []
                    if i + 2 < NI:
                        items.append((-2, lambda: issue_xq(i + 2, nc.sync)))
                    if i > 0:
                        ao_pair = [ao_tiles[(i - 1, 0)], ao_tiles[(i - 1, 1)]]
                        # last chunk: spread outproj(i-1) into the ACT-paced
                        # diagonal endgame instead of bunching it up front
                        slots = [-2, 8, 12, 14] if i == NI - 1 else [-2, -2, 2, 4]
                        for st in range(4):
                            items.append(
                                (slots[st], (lambda s: lambda: outproj_tile(i - 1, s, ao_pair))(st))
                            )
                    items.append((0, lambda: fins[(i, 0)]()))
                    if i == NI - 1:
                        st_box3 = {}

                        def g3():
                            st_box3["k"] = proj_group(i, 3)

                        def r3():
                            proj_rot(i, 3, st_box3["k"])

                        items += [(5, g3), (7, r3)]
                    if i < NI - 1:
                        st_box = {}

                        def g():
                            st_box["q"] = proj_group(i + 1, 0)

                        def r():
                            proj_rot(i + 1, 0, st_box["q"])

                        items += [(6 if i > 0 else -2, g), (7 if i > 0 else 1, r)]
                    return items

                for i in range(NI):
                    fins[(i, 0)] = make_attention(i, 0, hp0_items(i))
                    fins[(i, 1)] = make_attention(
                        i, 1, hp1_items(i), tail=(i == NI - 1)
                    )
            else:
                # non-causal: project everything first, then attention chunks
                # with outproj(i-1) interleaved.
                issue_xq(2, nc.sync)
                for q in range(NI):
                    if q == 2:
                        issue_xq(3, nc.sync)
                    rr = [proj_group(q, mt) for mt in range(4)]
                    for mt in range(4):
                        proj_rot(q, mt, rr[mt])
                    for st in range(4):
                        v_group(q, st)
                for i in range(NI):
                    items = []
                    if i > 0:
                        ao_pair = [ao_tiles[(i - 1, 0)], ao_tiles[(i - 1, 1)]]
                        for st in range(4):
                            items.append(
                                (st, (lambda s, p: lambda: outproj_tile(i - 1, s, p))(st, ao_pair))
                            )
                    make_attention(i, 0, items)()
                    make_attention(i, 1, [])()
                ao_pair = [ao_tiles[(NI - 1, 0)], ao_tiles[(NI - 1, 1)]]
                for st in range(4):
                    outproj_tile(NI - 1, st, ao_pair)

    nc.compile()
    return nc


# --------------------------------------------------------------------------
# host-side: constants, sharding, assembly
# --------------------------------------------------------------------------

def _rope_tables():
    inv_freq = 1.0 / (ROPE_BASE ** (np.arange(0, HD, 2, dtype=np.float32) / HD))
    positions = np.arange(S, dtype=np.float32)
    freqs = np.outer(positions, inv_freq).astype(np.float32)     # [S, 32]
    emb = np.concatenate((freqs, freqs), axis=-1)                # [S, 64]
    cosT = np.cos(emb).T.astype(np.float32)                      # [64, S]
    sinT = np.sin(emb).T.astype(np.float32)
    cos2 = np.vstack([cosT, cosT]).astype(ml_dtypes.bfloat16)    # [128, S]
    sin2 = np.vstack([sinT, sinT]).astype(ml_dtypes.bfloat16)
    return cos2, sin2


def _rot_lhsT():
    # rotate_half (interleaved): rot[2i] = -x[2i+1], rot[2i+1] = x[2i]
    P = np.zeros((128, 128), np.float32)
    for base in (0, 64):
        for i2 in range(HD // 2):
            P[base + 2 * i2, base + 2 * i2 + 1] = -1.0
            P[base + 2 * i2 + 1, base + 2 * i2] = 1.0
    return np.ascontiguousarray(P.T).astype(ml_dtypes.bfloat16)


_CACHE: dict = {}


def _get_nc(causal: bool):
    key = ("nc", causal)
    if key not in _CACHE:
        _CACHE[key] = build_nc(causal)
    return _CACHE[key]


def _classify_mask(mask: np.ndarray) -> str:
    m = np.asarray(mask).reshape(S, S)
    if np.array_equal(m != 0, np.tril(np.ones((S, S), bool))):
        return "causal"
    if np.all(m != 0):
        return "full"
    return "other"


def make_in_maps(x, w_qkv, w_out):
    """Build the 8 per-core input dicts."""
    cos2, sin2 = _rope_tables()
    rotP = _rot_lhsT()
    m01 = (np.arange(128)[:, None] <= np.arange(128)[None, :]).astype(np.float32)
    mask01 = np.ascontiguousarray(
        np.stack([m01, m01], axis=1).reshape(128, 256)
    ).astype(ml_dtypes.bfloat16)

    w3 = np.asarray(w_qkv).reshape(D, 3, H, HD)   # [D, {q,k,v}, H, hd]
    wo = np.asarray(w_out)                        # [D, D]; rows indexed [h, hd]
    xT = [
        np.ascontiguousarray(np.asarray(x)[b].T).astype(ml_dtypes.bfloat16)
        for b in range(B)
    ]  # [D, S]

    in_maps = []
    for c in range(8):
        b, hg = divmod(c, 4)
        hs = [4 * hg + i for i in range(HPC)]
        # w_qk cols: [q_h0, q_h1, k_h0, k_h1, q_h2, q_h3, k_h2, k_h3],
        # each head block column-permuted to the [evens; odds] layout
        wqk_cols = []
        for pair in range(2):
            for t in range(2):  # 0 = q, 1 = k
                for hh in (hs[2 * pair], hs[2 * pair + 1]):
                    wqk_cols.append(w3[:, t, hh, :])
        w_qk_c = np.ascontiguousarray(np.concatenate(wqk_cols, axis=1)).astype(
            ml_dtypes.bfloat16
        )  # [D, 512]
        w_v_c = np.ascontiguousarray(
            np.concatenate([w3[:, 2, hh, :] for hh in hs], axis=1)
        ).astype(ml_dtypes.bfloat16)  # [D, 256]
        w_out_c = np.ascontiguousarray(
            np.concatenate([wo[HD * hh : HD * hh + HD, :] for hh in hs], axis=0)
        ).astype(ml_dtypes.bfloat16)  # [256, D]
        in_maps.append(
            {
                "xT": xT[b],
                "w_qk": w_qk_c,
                "w_v": w_v_c,
                "w_out": w_out_c,
                "cos2": cos2,
                "sin2": sin2,
                "rotP": rotP,
                "mask01": mask01,
            }
        )
    return in_maps


def _reference_numpy(x, mask, w_qkv, w_out):
    """Exact fallback for non-causal, non-full masks (slow, host-side)."""
    x = np.asarray(x, np.float32)
    qkv = (x @ w_qkv).reshape(B, S, 3, H, HD)
    qkv = np.transpose(qkv, (2, 0, 3, 1, 4))
    q, k, v = qkv[0], qkv[1], qkv[2]
    inv_freq = 1.0 / (ROPE_BASE ** (np.arange(0, HD, 2, dtype=np.float32) / HD))
    freqs = np.outer(np.arange(S, dtype=np.float32), inv_freq)
    emb = np.concatenate((freqs, freqs), axis=-1)
    cos = np.cos(emb)[None, None]
    sin = np.sin(emb)[None, None]

    def rot(t):
        t1 = t[..., ::2]
        t2 = t[..., 1::2]
        return np.stack((-t2, t1), axis=-1).reshape(t.shape)

    q = q * cos + rot(q) * sin
    k = k * cos + rot(k) * sin
    attn = np.einsum("bhsd,bhtd->bhst", q, k) / math.sqrt(HD)
    m = np.asarray(mask).reshape(1, 1, S, S)
    attn = np.where(m == 0, -np.inf, attn)
    attn = attn - attn.max(-1, keepdims=True)
    np.exp(attn, out=attn)
    attn /= attn.sum(-1, keepdims=True)
    out = np.einsum("bhst,bhtd->bhsd", attn, v)
    out = np.transpose(out, (0, 2, 1, 3)).reshape(B, S, D)
    return (out @ w_out).astype(np.float32)


class Runner:
    """Cached jitted SPMD runner (mirrors bass2jax.run_bass_via_pjrt)."""

    def __init__(self, nc, n_cores: int = 8):
        import jax
        import concourse.mybir as _mybir
        from concourse import bass2jax
        from jax.experimental.shard_map import shard_map
        from jax.sharding import Mesh, PartitionSpec

        bass2jax.install_neuronx_cc_hook()
        self.jax = jax
        self.n_cores = n_cores
        self._nc = nc
        in_names, out_names, out_avals, zero_outs = [], [], [], []
        for alloc in nc.m.functions[0].allocations:
            if not isinstance(alloc, _mybir.MemoryLocationSet):
                continue
            name = alloc.memorylocations[0].name
            if alloc.kind == "ExternalInput":
                in_names.append(name)
            elif alloc.kind == "ExternalOutput":
                out_names.append(name)
                shape = tuple(alloc.tensor_shape)
                dtype = _mybir.dt.np(alloc.dtype)
                out_avals.append(jax.core.ShapedArray(shape, dtype))
                zero_outs.append(np.zeros(shape, dtype))
        self.in_names = list(in_names)
        self.out_names = out_names
        self.out_avals = out_avals
        self.zero_outs = zero_outs
        all_names = in_names + out_names

        def _body(*args):
            outs = bass2jax._bass_exec_p.bind(
                *args,
                out_avals=tuple(out_avals),
                in_names=tuple(all_names),
                out_names=tuple(out_names),
                lowering_input_output_aliases=(),
                sim_require_finite=True,
                sim_require_nnan=True,
                nc=nc,
            )
            return tuple(outs)

        devices = jax.devices()[:n_cores]
        self.mesh = Mesh(np.asarray(devices), ("core",))
        n_args = len(all_names)
        self.sharded = jax.jit(
            shard_map(
                _body,
                mesh=self.mesh,
                in_specs=(PartitionSpec("core"),) * n_args,
                out_specs=(PartitionSpec("core"),) * len(out_names),
                check_rep=False,
            )
        )

    def concat_inputs(self, in_maps):
        cols = []
        for name in self.in_names:
            if name == "partition_id":
                cols.append(
                    np.arange(self.n_cores, dtype=np.uint32).reshape(self.n_cores, 1)
                )
            else:
                cols.append(
                    np.concatenate([np.asarray(m[name]) for m in in_maps], axis=0)
                )
        return cols

    def device_put(self, concat_in):
        from jax.sharding import NamedSharding, PartitionSpec

        sh = NamedSharding(self.mesh, PartitionSpec("core"))
        args = concat_in + [
            np.zeros((self.n_cores * z.shape[0], *z.shape[1:]), z.dtype)
            for z in self.zero_outs
        ]
        return [self.jax.device_put(a, sh) for a in args]

    def run_dev(self, dev_args):
        return self.sharded(*dev_args)

    def make_bench(self, n_reps: int):
        import jax
        from concourse import bass2jax
        from jax.experimental.shard_map import shard_map
        from jax.sharding import Mesh, PartitionSpec

        nc = self._nc
        out_avals = self.out_avals
        all_names = self.in_names + self.out_names
        out_names = self.out_names

        def _body(*args):
            outs = None
            for _ in range(n_reps):
                outs = bass2jax._bass_exec_p.bind(
                    *args,
                    out_avals=tuple(out_avals),
                    in_names=tuple(all_names),
                    out_names=tuple(out_names),
                    lowering_input_output_aliases=(),
                    sim_require_finite=True,
                    sim_require_nnan=True,
                    nc=nc,
                )
            return tuple(outs)

        n_args = len(all_names)
        return jax.jit(
            shard_map(
                _body,
                mesh=self.mesh,
                in_specs=(PartitionSpec("core"),) * n_args,
                out_specs=(PartitionSpec("core"),) * len(out_names),
                check_rep=False,
            )
        )

    def run(self, in_maps):
        dev_args = self.device_put(self.concat_inputs(in_maps))
        out_arrs = self.sharded(*dev_args)
        outs = []
        for c in range(self.n_cores):
            outs.append(
                {
                    name: np.asarray(out_arrs[i]).reshape(
                        self.n_cores, *self.out_avals[i].shape
                    )[c]
                    for i, name in enumerate(self.out_names)
                }
            )
        return outs


def _get_runner(causal: bool) -> Runner:
    key = ("runner", causal)
    if key not in _CACHE:
        _CACHE[key] = Runner(_get_nc(causal))
    return _CACHE[key]


def run_spmd(in_maps, causal: bool = True, **kw):
    nc = _get_nc(causal)
    return run_bass_kernel_spmd(nc, in_maps, core_ids=list(range(8)), **kw)


def kernel(x, mask, w_qkv, w_out):
    kind = _classify_mask(mask)
    if kind == "other":
        return _reference_numpy(x, mask, w_qkv, w_out)
    in_maps = make_in_maps(x, w_qkv, w_out)
    res = run_spmd(in_maps, causal=(kind == "causal"))
    out = np.zeros((B, S, D), np.float32)
    for c in range(8):
        out[c // 4] += np.asarray(res.results[c]["outp"]).astype(np.float32)
    return out


if __name__ == "__main__":
    rng = np.random.default_rng(0)
    x = rng.standard_normal((B, S, D)).astype(np.float32)
    mask = np.tril(np.ones((S, S), np.int32)).reshape(1, 1, S, S)
    w_qkv = (rng.standard_normal((D, 3 * D)) * 0.02).astype(np.float32)
    w_out = (rng.standard_normal((D, D)) * 0.02).astype(np.float32)
    got = kernel(x, mask, w_qkv, w_out)
    print("kernel ran, out shape", got.shape)
